# revision 13
# baseline (speedup 1.0000x reference)
"""Trainium2 Bass kernel for nn_DilatedConvModel (retrieval_knn).

Model: eeg [B,T,64] -> 1x1 conv (64->8) -> dilated conv stack (8->16->16->16,
dilations 1,3,9, VALID, relu); stimulus [B,S,T,1] -> dilated stack
(1->16->16->16); cosine similarity between all stim/eeg channel pairs over
time; 256->1 linear.  B=64, S=8, T=8192.

Sharding: pure data parallel over B across 8 cores (8 sequences per core).

Per-core dataflow: channel-major convs on PE with block-diagonal weights
over the 8 local sequences.  e1/e2/e3/s2 run as fp8 DoubleRow matmuls with
hi+lo split fp8 weights (two k-tiles = (fp8(W), fp8(W-fp8(W))) applied to a
stride-0-replicated ifmap pair) at 0.5 cyc/row per tap; s1 and s3 stay
bf16 (fp8 quantization of raw stim / of s3's input costs too much accuracy
-- the fp8 activation noise gets correlated along t by the convs and stops
averaging out in the cosine).  Per-layer power-of-2 activation scales
(calibrated in numpy at const-build time) keep fp8 activations in range;
the scales cancel exactly in the cosine.  The dot contracts t in fp8 with
byte-paired DoubleRow: st/xf (fp8) are DMA-transposed as bitcast uint16
(pairs of adjacent t), the stationary xT is repacked to (even,odd) planes.
Final cosine/linear stays f32, norms are computed from the same fp8 values
so quantization cancels in the normalization.
"""

from contextlib import ExitStack

import numpy as np
import ml_dtypes

import concourse.bass as bass
import concourse.tile as tile
from concourse import mybir
from concourse.bass_utils import run_bass_kernel_spmd
from concourse.vector_clock import ScopedClock

# ---------------------------------------------------------------------------
# Workaround for walrus in this container rejecting >1 sync wait per
# instruction ("Too many sync wait commands").
# ---------------------------------------------------------------------------
_MAX_WAITS = 1


def _patched_drain_and_barrier(self, tick_clock, wait_clock):
    nc = self.nc
    probe = nc.sync.nop()
    wait_clock.add_sem_waits(probe.ins,
                             ScopedClock({None: tick_clock.global_clock}))
    si = probe.ins.sync_info
    waits = list(si.on_wait) if si and si.on_wait else []
    if len(waits) > _MAX_WAITS:
        si.on_wait = waits[:_MAX_WAITS]
        rest = waits[_MAX_WAITS:]
        while rest:
            extra = nc.sync.nop()
            extra.ins.sync_info = mybir.SyncInfo(on_wait=rest[:_MAX_WAITS],
                                                 on_update=[])
            rest = rest[_MAX_WAITS:]
    nc.sync.drain()
    nc.all_engine_barrier()
    assert self.sems is not None
    popped = nc._tile_sem_poison_stack.pop()
    assert popped is self._sem_poison
    nc.clear_and_free_semaphores(list(self.sems.allocated().values()))
    nc.all_engine_barrier()


def _split_multi_waits(nc, max_waits=_MAX_WAITS):
    f = nc.m.functions[0]
    ctr = 0
    for bb in f.blocks:
        new_insts = []
        for inst in bb.instructions:
            si = inst.sync_info
            waits = list(si.on_wait) if si and si.on_wait else []
            if len(waits) > max_waits:
                for w in waits[:-max_waits]:
                    ev = mybir.InstEventSemaphore(
                        name=f"waitsplit_{ctr}", opcode="EventSemaphore",
                        engine=inst.engine, ins=[], outs=[],
                        sync_info=mybir.SyncInfo(on_wait=[w], on_update=[]))
                    ctr += 1
                    new_insts.append(ev)
                si.on_wait = waits[-max_waits:]
            new_insts.append(inst)
        try:
            bb.instructions[:] = new_insts
        except TypeError:
            bb.instructions = new_insts


tile.TileContext._drain_and_barrier = _patched_drain_and_barrier

BF16 = mybir.dt.bfloat16
F32 = mybir.dt.float32
FP8 = mybir.dt.float8e4
U16 = mybir.dt.uint16
AF = mybir.ActivationFunctionType
ALU = mybir.AluOpType
DR = mybir.MatmulPerfMode.DoubleRow

B, S, T, C_EEG = 64, 8, 8192, 64
N_CORES = 8
BPC = B // N_CORES          # 8 sequences per core
CH = 512                    # fp32 PSUM chunk width
L_C1, L_E1, L_E2, L_E3 = 8192, 8190, 8184, 8166
EPS = 1e-8
NPAIR = T // 2              # 4096 u16 t-pairs
NCHK = NPAIR // 128         # 32 dot chunks

_NC_CACHE = {}


def _chunks(length):
    out, t0 = [], 0
    while t0 < length:
        w = min(CH, length - t0)
        out.append((t0, w))
        t0 += w
    return out


def _const_shapes():
    d = {
        "Ws1": ((24, 128), BF16),
        "id128": ((128, 128), BF16),
        "id128f": ((128, 128), F32),
        "W2c": ((128, 128), F32),
        "SEL": ((128, 8), F32),
        "ones1x128": ((1, 128), F32),
        "blin": ((128, 1), F32),
    }
    for k in range(3):
        for lp in range(4):
            d[f"We1_{k}_{lp}"] = ((128, 2, 128), FP8)   # padded per lp
    for l in (2, 3):
        for k in range(3):
            d[f"We{l}_{k}"] = ((128, 2, 128), FP8)
    for k in range(3):
        d[f"Ws2_{k}"] = ((128, 2, 128), FP8)
        d[f"Ws3_{k}"] = ((128, 128), BF16)
    for n in ("bias_e1", "bias_e2", "bias_e3", "bias_s1", "bias_s2",
              "bias_s3"):
        d[n] = ((128, 1), F32)
    return d


def _blob_layout():
    """column layout of consts inside the three dtype blobs"""
    items = {"bf": [], "f32": [], "fp8": []}
    for name, (shape, dt) in _const_shapes().items():
        which = {BF16: "bf", F32: "f32", FP8: "fp8"}[dt]
        items[which].append((name, shape))
    lay, off = {}, {"bf": 0, "f32": 0, "fp8": 0}
    for which, lst in items.items():
        for name, shape in lst:
            w = int(np.prod(shape[1:]))
            lay[name] = (which, off[which], shape)
            off[which] += w
    return lay, off["bf"], off["f32"], off["fp8"]


def _build_body(nc, tc, dram):
    eeg_in, stim_in, out_dram = dram["eeg_in"], dram["stim_in"], dram["out"]

    with ExitStack() as ctx:
        const_p = ctx.enter_context(tc.tile_pool(name="const", bufs=1))
        persist_p = ctx.enter_context(tc.tile_pool(name="persist", bufs=1))
        early_p = ctx.enter_context(tc.tile_pool(name="early", bufs=1))
        psC_p = ctx.enter_context(tc.tile_pool(name="psC", bufs=3,
                                               space="PSUM"))

        lay, nbf, nf, n8 = _blob_layout()
        blob_bf = const_p.tile([128, nbf], BF16, name="blob_bf")
        nc.sync.dma_start(blob_bf[:], dram["blob_bf"][:])
        blob_f32 = const_p.tile([128, nf], F32, name="blob_f32")
        nc.sync.dma_start(blob_f32[:], dram["blob_f32"][:])
        blob_fp8 = const_p.tile([128, n8], FP8, name="blob_fp8")
        nc.sync.dma_start(blob_fp8[:], dram["blob_fp8"][:])

        def cload(name):
            which, off, shape = lay[name]
            blob = {"bf": blob_bf, "f32": blob_f32, "fp8": blob_fp8}[which]
            w = int(np.prod(shape[1:]))
            ap = blob[0:shape[0], off:off + w]
            if len(shape) == 3:
                ap = ap.rearrange("p (a b) -> p a b", a=shape[1])
            return ap

        We1 = {(k, lp): cload(f"We1_{k}_{lp}")
               for k in range(3) for lp in range(4)}
        We = {l: [cload(f"We{l}_{k}") for k in range(3)] for l in (2, 3)}
        Ws1 = cload("Ws1")
        Ws2 = [cload(f"Ws2_{k}") for k in range(3)]
        Ws3 = [cload(f"Ws3_{k}") for k in range(3)]
        id128 = cload("id128")
        id128f = cload("id128f")
        bias = {n: cload(n) for n in
                ("bias_e1", "bias_e2", "bias_e3",
                 "bias_s1", "bias_s2", "bias_s3")}
        W2c = cload("W2c")
        SEL = cload("SEL")
        ones1x128 = cload("ones1x128")
        blin = cload("blin")

        out_sb = const_p.tile([1, BPC * S], F32, name="out_sb")
        inv_nx = const_p.tile([128, 1], F32, name="inv_nx")
        sqscr = const_p.tile([128, T], BF16, name="sqscr")

        xf = persist_p.tile([128, T], FP8, name="xf")
        xT2 = persist_p.tile([128, NCHK, 2, 128], FP8, name="xT2")

        evac_ctr = [0]

        phase = ["stim"]

        def evac_relu(dst, src, bias_t):
            # eeg phase: ACT handles psT copies, so conv evacs go to DVE;
            # stim phase: norms run on ACT, split evacs 3:2 DVE:ACT
            if phase[0] == "eeg":
                use_dve = True
            else:
                use_dve = evac_ctr[0] % 5 < 3
            if use_dve:
                nc.vector.tensor_scalar(dst, src, bias_t[:, 0:1], 0.0,
                                        ALU.add, ALU.max)
            else:
                nc.scalar.activation(dst, src, AF.Relu, bias=bias_t[:, 0:1])
            evac_ctr[0] += 1

        def pair0(ap):
            """stride-0 k-tile pair view of a 2D ifmap slice"""
            v = ap.unsqueeze(1)
            v.ap[1] = [0, 2]
            return v

        def conv_dr(src_m, dst_m, out_len, dil, Wk, bn):
            # fp8 DoubleRow conv: per tap one DR matmul with (hi, lo) weight
            # tiles and a stride-0 ifmap pair; 0.5 cyc/row per tap.
            chs = _chunks(out_len)
            for i in range(0, len(chs), 2):
                grp = chs[i:i + 2]
                ps = psC_p.tile([128, 2 * CH], F32, name="psconv",
                                tag="psconv")
                for k in range(3):
                    for j, (t0, w) in enumerate(grp):
                        nc.tensor.matmul(
                            ps[:, j * CH:j * CH + w], Wk[k],
                            pair0(src_m[0:128,
                                        t0 + k * dil:t0 + k * dil + w]),
                            start=(k == 0), stop=(k == 2), perf_mode=DR)
                t0 = grp[0][0]
                wtot = CH + grp[1][1] if len(grp) == 2 else grp[0][1]
                evac_relu(dst_m[:, t0:t0 + wtot], ps[:, :wtot], bias[bn])

        def conv_bf16(src_m, dst_m, out_len, dil, Wk, bn):
            chs = _chunks(out_len)
            for i in range(0, len(chs), 2):
                grp = chs[i:i + 2]
                ps = psC_p.tile([128, 2 * CH], F32, name="psconv",
                                tag="psconv")
                for k in range(3):
                    for j, (t0, w) in enumerate(grp):
                        nc.tensor.matmul(
                            ps[:, j * CH:j * CH + w], Wk[k],
                            src_m[0:128, t0 + k * dil:t0 + k * dil + w],
                            start=(k == 0), stop=(k == 2))
                t0 = grp[0][0]
                wtot = CH + grp[1][1] if len(grp) == 2 else grp[0][1]
                evac_relu(dst_m[:, t0:t0 + wtot], ps[:, :wtot], bias[bn])

        # ---- early: stimulus group 0 s1+s2 (fills PE while eeg DMA runs)
        s1movs = [early_p.tile([24, T], BF16, name="s1mov",
                               tag=f"s1mov{i}") for i in range(2)]
        s2in = early_p.tile([128, L_E1], FP8, name="s2in")
        s3ins = [early_p.tile([128, L_E2], BF16, name="s3in",
                              tag=f"s3in{i}") for i in range(2)]

        def stim_s1(g):
            s1mov = s1movs[g % 2]
            for k in range(3):
                nc.gpsimd.dma_start(s1mov[k * 8:(k + 1) * 8, 0:L_E1],
                                    stim_in[g, :, k:k + L_E1])
            chs = _chunks(L_E1)
            for i in range(0, len(chs), 2):
                grp = chs[i:i + 2]
                ps = psC_p.tile([128, 2 * CH], F32, name="psconv",
                                tag="psconv")
                for j, (t0, w) in enumerate(grp):
                    nc.tensor.matmul(ps[:, j * CH:j * CH + w], Ws1[:],
                                     s1mov[0:24, t0:t0 + w])
                t0 = grp[0][0]
                wtot = CH + grp[1][1] if len(grp) == 2 else grp[0][1]
                evac_relu(s2in[:, t0:t0 + wtot], ps[:, :wtot],
                          bias["bias_s1"])

        stim_s1(0)
        conv_dr(s2in, s3ins[0], L_E2, 3, Ws2, "bias_s2")
        stim_s1(1)
        conv_dr(s2in, s3ins[1], L_E2, 3, Ws2, "bias_s2")

        # ================= EEG path =================
        with ExitStack() as ectx:
            phase[0] = "eeg"
            eeg_p = ectx.enter_context(tc.tile_pool(name="eegp", bufs=1))
            rot_p = ectx.enter_context(tc.tile_pool(name="eegrot", bufs=3))
            psT_p = ectx.enter_context(tc.tile_pool(name="psT", bufs=2,
                                                    space="PSUM"))

            e2in = eeg_p.tile([128, L_E1], FP8, name="e2in")
            e3in = eeg_p.tile([128, L_E2], FP8, name="e3in")

            TB = 4096
            chs_e1 = _chunks(L_E1)
            for duo in range(2):
                eegT = {}
                for lp in range(2):
                    p = 2 * duo + lp
                    eegT_p = eeg_p.tile([128, T], FP8, name="eegT",
                                        tag=f"eegT_{lp}")
                    eegT[lp] = eegT_p
                    for tb in range(T // TB):
                        ebf = rot_p.tile([128, TB // 128, 2, 64], BF16,
                                         name="ebf")
                        for dlt in range(2):
                            srcd = eeg_in[2 * p + dlt,
                                          tb * TB:(tb + 1) * TB, :]
                            nc.gpsimd.dma_start(
                                ebf[:, :, dlt, :],
                                srcd.rearrange("(th tl) c -> tl th c",
                                               tl=128))
                        for qb in range(TB // (2 * CH)):
                            psT = psT_p.tile([128, 8, 128], BF16,
                                             name="psT")
                            for u in range(8):
                                nc.tensor.matmul(psT[:, u, :],
                                                 ebf[:, qb * 8 + u, :, :],
                                                 id128[:],
                                                 is_transpose=True,
                                                 start=(u == 0),
                                                 stop=(u == 7))
                            t0 = tb * TB + qb * 2 * CH
                            nc.scalar.copy(
                                eegT_p[:, t0:t0 + 2 * CH], psT[:])
                # fused conv1x1+e1 in fp8 DR with padded 128-col weights;
                # lp outputs land at psum rows 32*(2*duo+lp)
                for i in range(0, len(chs_e1), 2):
                    grp = chs_e1[i:i + 2]
                    t0 = grp[0][0]
                    wtot = CH + grp[1][1] if len(grp) == 2 else grp[0][1]
                    ps = psC_p.tile([128, 2 * CH], F32, name="pse1",
                                    tag="psconv")
                    for lp in range(2):
                        for k in range(3):
                            for j, (tj, w) in enumerate(grp):
                                nc.tensor.matmul(
                                    ps[:, j * CH:j * CH + w],
                                    We1[(k, 2 * duo + lp)],
                                    pair0(eegT[lp][:, tj + k:tj + k + w]),
                                    start=(lp == 0 and k == 0),
                                    stop=(lp == 1 and k == 2),
                                    perf_mode=DR)
                    r0 = 64 * duo
                    evac_relu(e2in[r0:r0 + 64, t0:t0 + wtot],
                              ps[r0:r0 + 64, :wtot],
                              bias["bias_e1"][r0:r0 + 64])

            conv_dr(e2in, e3in, L_E2, 3, We[2], "bias_e2")
            conv_dr(e3in, xf, L_E3, 9, We[3], "bias_e3")
            phase[0] = "stim"

        # ================= stimulus path =================
        with ExitStack() as sctx:
            stim_p = sctx.enter_context(tc.tile_pool(name="stimp", bufs=1))
            stT_p = sctx.enter_context(tc.tile_pool(name="stTp", bufs=2))
            psD_p = sctx.enter_context(tc.tile_pool(name="psD", bufs=1,
                                                    space="PSUM"))
            psF_p = sctx.enter_context(tc.tile_pool(name="psF", bufs=1,
                                                    space="PSUM"))

            pending = []

            def emit_dot(g, stT, invns_row):
                dot_ps = psD_p.tile([128, 128], F32, name="dot_ps",
                                    tag="dot_ps")
                for c in range(NCHK):
                    rv = stT[:, c, :].bitcast(FP8).unsqueeze(1)
                    rv.ap[1] = [1, 2]
                    rv.ap[2] = [2, 128]
                    nc.tensor.matmul(dot_ps[:], xT2[:, c, :, :], rv,
                                     start=(c == 0), stop=(c == NCHK - 1),
                                     perf_mode=DR)
                # inv_ns broadcast over all partitions via two tiny matmuls
                f1 = const_p.tile([128, 128], F32, name="f1",
                                  tag=f"f1_{g % 2}")
                nc.vector.tensor_mul(f1[:], dot_ps[:], W2c[:])
                # psB reuses dot_ps's bank (WAR-ordered after the mul above)
                psB = psD_p.tile([128, 128], F32, name="psB", tag="dot_ps")
                nc.tensor.matmul(psB[:], ones1x128[:], invns_row[:])
                nc.vector.tensor_mul(f1[:], f1[:], psB[:])
                nc.vector.tensor_scalar_mul(f1[:], f1[:], inv_nx[:, 0:1])
                f3 = const_p.tile([128, S], F32, name="f3",
                                  tag=f"f3_{g % 2}")
                nc.vector.tensor_reduce(
                    f3[:], f1.rearrange("p (s i) -> p s i", i=16),
                    mybir.AxisListType.X, ALU.add)
                fin_ps = psF_p.tile([1, S], F32, name="fin_ps", tag="psN")
                nc.tensor.matmul(fin_ps[:], SEL[:, g:g + 1], f3[:])
                nc.vector.tensor_scalar_add(
                    out_sb[0:1, g * S:(g + 1) * S], fin_ps[:],
                    blin[0:1, 0:1])

            for g in range(BPC):
                st_cm = stim_p.tile([128, T], FP8, name="st_cm", bufs=2)
                nc.gpsimd.memset(st_cm[:, L_E3:T], 0.0)
                conv_bf16(s3ins[g % 2], st_cm, L_E3, 9, Ws3, "bias_s3")
                if g + 2 < BPC:
                    stim_s1(g + 2)
                    conv_dr(s2in, s3ins[g % 2], L_E2, 3, Ws2, "bias_s2")
                if g == 0:
                    # x norms from fp8 xf + packed-pair transpose + repack
                    nx2 = const_p.tile([128, 1], F32, name="nx2")
                    nx4 = const_p.tile([128, 4], F32, name="nx4")
                    qs = 2048
                    for q in range(4):
                        a, b = q * qs, min((q + 1) * qs, L_E3)
                        nc.scalar.activation(sqscr[:, a:b], xf[:, a:b],
                                             AF.Square,
                                             accum_out=nx4[:, q:q + 1])
                    nc.vector.tensor_reduce(nx2[:], nx4[:],
                                            mybir.AxisListType.X, ALU.add)
                    nc.scalar.sqrt(inv_nx[:], nx2[:])
                    nc.vector.tensor_scalar_max(inv_nx[:], inv_nx[:], EPS)
                    nc.vector.reciprocal(inv_nx[:], inv_nx[:])
                    nc.gpsimd.memset(xf[:, L_E3:T], 0.0)
                    xTu = stim_p.tile([128, NCHK, 128], U16, name="xTu")
                    for hh in range(2):
                        nc.sync.dma_start_transpose(
                            xTu[:, hh * 16:(hh + 1) * 16, :],
                            xf[:, hh * (T // 2):(hh + 1) * (T // 2)]
                            .bitcast(U16))
                    # repack byte pairs -> (even, odd) planes for ldweights
                    src = xTu[:].bitcast(FP8).unsqueeze(2)
                    src.ap[2] = [1, 2]
                    src.ap[3] = [2, 128]
                    nc.vector.tensor_copy(xT2[:], src)

                ns2 = const_p.tile([128, 1], F32, name="ns2",
                                   tag=f"ns2_{g % 2}")
                ns4 = const_p.tile([128, 4], F32, name="ns4",
                                   tag=f"ns4_{g % 2}")
                qs = 2048
                for q in range(4):
                    a, b = q * qs, min((q + 1) * qs, L_E3)
                    nc.scalar.activation(sqscr[:, a:b], st_cm[:, a:b],
                                         AF.Square,
                                         accum_out=ns4[:, q:q + 1])
                nc.vector.tensor_reduce(ns2[:], ns4[:],
                                        mybir.AxisListType.X, ALU.add)
                inv_ns = const_p.tile([128, 1], F32, name="inv_ns",
                                      tag=f"invns_{g % 2}")
                nc.scalar.sqrt(inv_ns[:], ns2[:])
                nc.vector.tensor_scalar_max(inv_ns[:], inv_ns[:], EPS)
                nc.vector.reciprocal(inv_ns[:], inv_ns[:])
                psN = psF_p.tile([1, 128], F32, name="psN", tag="psN")
                nc.tensor.matmul(psN[:], inv_ns[:], id128f[:],
                                 is_transpose=True)
                invns_row = const_p.tile([1, 128], F32, name="invns_row",
                                         tag=f"invrow_{g % 2}")
                nc.vector.tensor_copy(invns_row[:], psN[:])

                stT = stT_p.tile([128, NCHK, 128], U16, name="stT")
                for hh in range(2):
                    nc.sync.dma_start_transpose(
                        stT[:, hh * 16:(hh + 1) * 16, :],
                        st_cm[:, hh * (T // 2):(hh + 1) * (T // 2)]
                        .bitcast(U16))
                pending.append((g, stT, invns_row))
                if len(pending) > 1:
                    emit_dot(*pending.pop(0))
            while pending:
                emit_dot(*pending.pop(0))

        nc.sync.dma_start(out_dram[:], out_sb[:])


def _build(reps=1):
    nc = bass.Bass()
    dram = {
        "eeg_in": nc.dram_tensor("eeg_in", [BPC, T, C_EEG], F32,
                                 kind="ExternalInput"),
        "stim_in": nc.dram_tensor("stim_in", [BPC, S, T], F32,
                                  kind="ExternalInput"),
    }
    lay, nbf, nf, n8 = _blob_layout()
    dram["blob_bf"] = nc.dram_tensor("blob_bf", [128, nbf], BF16,
                                     kind="ExternalInput")
    dram["blob_f32"] = nc.dram_tensor("blob_f32", [128, nf], F32,
                                      kind="ExternalInput")
    dram["blob_fp8"] = nc.dram_tensor("blob_fp8", [128, n8], FP8,
                                      kind="ExternalInput")
    dram["out"] = nc.dram_tensor("out", [1, BPC * S], F32,
                                 kind="ExternalOutput")

    with tile.TileContext(nc) as tc:
        _build_body(nc, tc, dram)
    _split_multi_waits(nc)
    return nc


def _calib_scales(inp):
    """flat power-of-2 per-layer activation scales from a numpy calibration
    pass over a slice of the real inputs (outputs sampled strided)."""
    f32 = np.float32

    def conv_np(x, w, b, dil):
        K = w.shape[2]
        L = x.shape[2] - dil * (K - 1)
        out = np.zeros((x.shape[0], w.shape[0], L), f32)
        for k in range(K):
            out += np.einsum('oc,nct->not', w[:, :, k].astype(f32),
                             x[:, :, k * dil:k * dil + L])
        return np.maximum(out + b[None, :, None], 0)

    def pow2(x):
        return float(2.0 ** np.round(np.log2(max(x, 1e-30))))

    TGT = 64.0
    sl = np.s_[:, :, ::4]
    eeg = np.asarray(inp['eeg'], f32)
    stim = np.asarray(inp['stimulus'], f32)[..., 0]
    w_eeg = np.asarray(inp['w_eeg'], f32)
    w_e1 = np.asarray(inp['w_e1'], f32)
    Wf1 = np.einsum('ock,ci->oik', w_e1, w_eeg[:, :, 0])
    b_e1f = (np.asarray(inp['b_e1'], f32) +
             w_e1.sum(2) @ np.asarray(inp['b_eeg'], f32))
    g = lambda n: np.asarray(inp[n], f32)

    a = conv_np(eeg.transpose(0, 2, 1)[:4], Wf1, b_e1f, 1)
    S1 = pow2(TGT / (np.abs(a[sl]).max() + 1e-12))
    a = conv_np(a * S1, g('w_e2') / S1, g('b_e2'), 3)
    S2 = pow2(TGT / (np.abs(a[sl]).max() + 1e-12))
    a = conv_np(a * S2, g('w_e3') / S2, g('b_e3'), 9)
    S3 = pow2(TGT / (np.abs(a[sl]).max() + 1e-12))
    a = conv_np(stim.reshape(B * S, 1, T)[:8], g('w_s1'), g('b_s1'), 1)
    T1 = pow2(TGT / (np.abs(a[sl]).max() + 1e-12))
    a = conv_np(a * T1, g('w_s2') / T1, g('b_s2'), 3)
    T2 = pow2(TGT / (np.abs(a[sl]).max() + 1e-12))
    a = conv_np(a * T2, g('w_s3') / T2, g('b_s3'), 9)
    T3 = pow2(TGT / (np.abs(a[sl]).max() + 1e-12))
    return S1, S2, S3, T1, T2, T3


def _make_consts(inp):
    bf = ml_dtypes.bfloat16
    f8 = ml_dtypes.float8_e4m3fn
    f32 = np.float32
    S1, S2, S3, T1, T2, T3 = _calib_scales(inp)
    c = {}
    w_eeg = np.asarray(inp["w_eeg"], f32)      # [8, 64, 1]
    w_e1 = np.asarray(inp["w_e1"], f32)

    def hi_lo(m):
        """[rows, cols] f32 -> [rows, 2, cols] fp8 (hi, residual lo)"""
        hi = np.clip(m, -448, 448).astype(f8).astype(f32)
        lo = np.clip(m - hi, -448, 448).astype(f8)
        return np.stack([hi.astype(f8), lo], axis=1)

    def blockdiag(w, n_seq, ci, co):
        out = []
        for k in range(3):
            m = np.zeros((n_seq * ci, n_seq * co), f32)
            for s in range(n_seq):
                m[s * ci:(s + 1) * ci, s * co:(s + 1) * co] = w[:, :, k].T
            out.append(m)
        return out

    # fused conv1x1 + e1 (scale S1), per-lp padded to 128 cols
    for k in range(3):
        Mk = (w_e1[:, :, k] @ w_eeg[:, :, 0]) * S1   # [16 co, 64 c]
        base = np.zeros((128, 32), f32)
        for s in range(2):
            base[s * 64:(s + 1) * 64, s * 16:(s + 1) * 16] = Mk.T
        for lp in range(4):
            m = np.zeros((128, 128), f32)
            m[:, 32 * lp:32 * lp + 32] = base
            c[f"We1_{k}_{lp}"] = hi_lo(m)
    for l, wn, sc in ((2, "w_e2", S2 / S1), (3, "w_e3", S3 / S2)):
        mats = blockdiag(np.asarray(inp[wn], f32) * sc, 8, 16, 16)
        for k in range(3):
            c[f"We{l}_{k}"] = hi_lo(mats[k])
    mats = blockdiag(np.asarray(inp["w_s2"], f32) * (T2 / T1), 8, 16, 16)
    for k in range(3):
        c[f"Ws2_{k}"] = hi_lo(mats[k])
    mats = blockdiag(np.asarray(inp["w_s3"], f32) * (T3 / T2), 8, 16, 16)
    for k in range(3):
        c[f"Ws3_{k}"] = mats[k].astype(bf)
    w_s1 = np.asarray(inp["w_s1"], f32) * T1     # [16, 1, 3]
    Ws1 = np.zeros((24, 128), f32)
    for k in range(3):
        for s in range(8):
            Ws1[k * 8 + s, s * 16:(s + 1) * 16] = w_s1[:, 0, k]
    c["Ws1"] = Ws1.astype(bf)
    c["id128"] = np.eye(128, dtype=f32).astype(bf)
    c["id128f"] = np.eye(128, dtype=f32)
    b_e1f = (np.asarray(inp["b_e1"], f32) +
             sum(w_e1[:, :, k] for k in range(3))
             @ np.asarray(inp["b_eeg"], f32)) * S1
    c["bias_e1"] = np.tile(b_e1f, 8)[:, None]
    for n, srcn, sc in (("bias_e2", "b_e2", S2), ("bias_e3", "b_e3", S3),
                        ("bias_s1", "b_s1", T1), ("bias_s2", "b_s2", T2),
                        ("bias_s3", "b_s3", T3)):
        c[n] = (np.tile(np.asarray(inp[srcn], f32), 8) * sc)[:, None]
    w_lin = np.asarray(inp["w_lin"], f32).reshape(16, 16)  # [i, j]
    W2c = np.zeros((128, 128), f32)
    for gp in range(8):
        for s in range(8):
            W2c[gp * 16:(gp + 1) * 16, s * 16:(s + 1) * 16] = w_lin.T
    c["W2c"] = W2c
    SEL = np.zeros((128, 8), f32)
    for gp in range(8):
        SEL[gp * 16:(gp + 1) * 16, gp] = 1.0
    c["SEL"] = SEL
    c["ones1x128"] = np.ones((1, 128), f32)
    c["blin"] = np.full((128, 1), np.asarray(inp["b_lin"], f32).ravel()[0],
                        f32)
    lay, nbf, nf, n8 = _blob_layout()
    bb = np.zeros((128, nbf), bf)
    bf32 = np.zeros((128, nf), f32)
    b8 = np.zeros((128, n8), f8)
    for name, (which, off, shape) in lay.items():
        dst = {"bf": bb, "f32": bf32, "fp8": b8}[which]
        w = int(np.prod(shape[1:]))
        dst[0:shape[0], off:off + w] = \
            np.asarray(c[name]).reshape(shape[0], w)
    return {"blob_bf": bb, "blob_f32": bf32, "blob_fp8": b8}


def get_nc(reps=1):
    if reps not in _NC_CACHE:
        _NC_CACHE[reps] = _build(reps)
    return _NC_CACHE[reps]


def run(inputs, reps=1, trace=False):
    nc = get_nc(reps)
    consts = _make_consts(inputs)
    eeg = np.asarray(inputs["eeg"], np.float32)              # [64, 8192, 64]
    stim = np.asarray(inputs["stimulus"], np.float32)[..., 0]  # [64, 8, 8192]
    in_maps = []
    for ci in range(N_CORES):
        m = {"eeg_in": np.ascontiguousarray(eeg[ci * BPC:(ci + 1) * BPC]),
             "stim_in": np.ascontiguousarray(stim[ci * BPC:(ci + 1) * BPC])}
        m.update(consts)
        in_maps.append(m)
    res = run_bass_kernel_spmd(nc, in_maps, list(range(N_CORES)),
                               trace=trace)
    out = np.concatenate(
        [res.results[i]["out"].reshape(BPC, S) for i in range(N_CORES)],
        axis=0)
    return out.astype(np.float32)


def kernel(**inputs):
    return run(inputs, reps=1)


# revision 20
# speedup vs baseline: 1.0251x; 1.0251x over previous
"""Trainium2 Bass kernel for nn_DilatedConvModel (retrieval_knn).

Model: eeg [B,T,64] -> 1x1 conv (64->8) -> dilated conv stack (8->16->16->16,
dilations 1,3,9, VALID, relu); stimulus [B,S,T,1] -> dilated stack
(1->16->16->16); cosine similarity between all stim/eeg channel pairs over
time; 256->1 linear.  B=64, S=8, T=8192.

Sharding: pure data parallel over B across 8 cores (8 sequences per core).

Per-core dataflow: channel-major convs on PE with block-diagonal weights
over the 8 local sequences.  e1/e2/e3/s2 run as fp8 DoubleRow matmuls with
hi+lo split fp8 weights (two k-tiles = (fp8(W), fp8(W-fp8(W))) applied to a
stride-0-replicated ifmap pair) at 0.5 cyc/row per tap; s1 and s3 stay
bf16 (fp8 quantization of raw stim / of s3's input costs too much accuracy
-- the fp8 activation noise gets correlated along t by the convs and stops
averaging out in the cosine).  Per-layer power-of-2 activation scales
(calibrated in numpy at const-build time) keep fp8 activations in range;
the scales cancel exactly in the cosine.  The dot contracts t in fp8 with
byte-paired DoubleRow: st/xf (fp8) are DMA-transposed as bitcast uint16
(pairs of adjacent t), the stationary xT is repacked to (even,odd) planes.
Final cosine/linear stays f32, norms are computed from the same fp8 values
so quantization cancels in the normalization.
"""

from contextlib import ExitStack

import numpy as np
import ml_dtypes

import concourse.bass as bass
import concourse.tile as tile
from concourse import mybir
from concourse.bass_utils import run_bass_kernel_spmd
from concourse.vector_clock import ScopedClock

# ---------------------------------------------------------------------------
# Workaround for walrus in this container rejecting >1 sync wait per
# instruction ("Too many sync wait commands").
# ---------------------------------------------------------------------------
_MAX_WAITS = 1


def _patched_drain_and_barrier(self, tick_clock, wait_clock):
    nc = self.nc
    probe = nc.sync.nop()
    wait_clock.add_sem_waits(probe.ins,
                             ScopedClock({None: tick_clock.global_clock}))
    si = probe.ins.sync_info
    waits = list(si.on_wait) if si and si.on_wait else []
    if len(waits) > _MAX_WAITS:
        si.on_wait = waits[:_MAX_WAITS]
        rest = waits[_MAX_WAITS:]
        while rest:
            extra = nc.sync.nop()
            extra.ins.sync_info = mybir.SyncInfo(on_wait=rest[:_MAX_WAITS],
                                                 on_update=[])
            rest = rest[_MAX_WAITS:]
    nc.sync.drain()
    nc.all_engine_barrier()
    assert self.sems is not None
    popped = nc._tile_sem_poison_stack.pop()
    assert popped is self._sem_poison
    nc.clear_and_free_semaphores(list(self.sems.allocated().values()))
    nc.all_engine_barrier()


def _split_multi_waits(nc, max_waits=_MAX_WAITS):
    f = nc.m.functions[0]
    ctr = 0
    for bb in f.blocks:
        new_insts = []
        for inst in bb.instructions:
            si = inst.sync_info
            waits = list(si.on_wait) if si and si.on_wait else []
            if len(waits) > max_waits:
                for w in waits[:-max_waits]:
                    ev = mybir.InstEventSemaphore(
                        name=f"waitsplit_{ctr}", opcode="EventSemaphore",
                        engine=inst.engine, ins=[], outs=[],
                        sync_info=mybir.SyncInfo(on_wait=[w], on_update=[]))
                    ctr += 1
                    new_insts.append(ev)
                si.on_wait = waits[-max_waits:]
            new_insts.append(inst)
        try:
            bb.instructions[:] = new_insts
        except TypeError:
            bb.instructions = new_insts


tile.TileContext._drain_and_barrier = _patched_drain_and_barrier

BF16 = mybir.dt.bfloat16
F32 = mybir.dt.float32
FP8 = mybir.dt.float8e4
U16 = mybir.dt.uint16
AF = mybir.ActivationFunctionType
ALU = mybir.AluOpType
DR = mybir.MatmulPerfMode.DoubleRow

B, S, T, C_EEG = 64, 8, 8192, 64
N_CORES = 8
BPC = B // N_CORES          # 8 sequences per core
CH = 512                    # fp32 PSUM chunk width
L_C1, L_E1, L_E2, L_E3 = 8192, 8190, 8184, 8166
EPS = 1e-8
NPAIR = T // 2              # 4096 u16 t-pairs
NCHK = NPAIR // 128         # 32 dot chunks

_NC_CACHE = {}


def _chunks(length):
    out, t0 = [], 0
    while t0 < length:
        w = min(CH, length - t0)
        out.append((t0, w))
        t0 += w
    return out


def _const_shapes():
    d = {
        "Ws1": ((24, 128), BF16),
        "id128": ((128, 128), BF16),
        "id128f": ((128, 128), F32),
        "W2c": ((128, 128), F32),
        "SEL": ((128, 8), F32),
        "ones1x128": ((1, 128), F32),
        "blin": ((128, 1), F32),
    }
    for k in range(3):
        for lp in range(4):
            d[f"We1_{k}_{lp}"] = ((128, 2, 128), FP8)   # padded per lp
    for l in (2, 3):
        for k in range(3):
            d[f"We{l}_{k}"] = ((128, 2, 128), FP8)
    for k in range(3):
        d[f"Ws2_{k}"] = ((128, 2, 128), FP8)
        d[f"Ws3_{k}"] = ((128, 128), BF16)
    for n in ("bias_e1", "bias_e2", "bias_e3", "bias_s1", "bias_s2",
              "bias_s3"):
        d[n] = ((128, 1), F32)
    return d


def _blob_layout():
    """column layout of consts inside the three dtype blobs"""
    items = {"bf": [], "f32": [], "fp8": []}
    for name, (shape, dt) in _const_shapes().items():
        which = {BF16: "bf", F32: "f32", FP8: "fp8"}[dt]
        items[which].append((name, shape))
    lay, off = {}, {"bf": 0, "f32": 0, "fp8": 0}
    for which, lst in items.items():
        for name, shape in lst:
            w = int(np.prod(shape[1:]))
            lay[name] = (which, off[which], shape)
            off[which] += w
    return lay, off["bf"], off["f32"], off["fp8"]


def _build_body(nc, tc, dram):
    eeg_in, stim_in, out_dram = dram["eeg_in"], dram["stim_in"], dram["out"]

    with ExitStack() as ctx:
        const_p = ctx.enter_context(tc.tile_pool(name="const", bufs=1))
        persist_p = ctx.enter_context(tc.tile_pool(name="persist", bufs=1))
        early_p = ctx.enter_context(tc.tile_pool(name="early", bufs=1))
        psC_p = ctx.enter_context(tc.tile_pool(name="psC", bufs=3,
                                               space="PSUM"))

        lay, nbf, nf, n8 = _blob_layout()
        blob_bf = const_p.tile([128, nbf], BF16, name="blob_bf")
        nc.sync.dma_start(blob_bf[:], dram["blob_bf"][:])
        blob_f32 = const_p.tile([128, nf], F32, name="blob_f32")
        nc.sync.dma_start(blob_f32[:], dram["blob_f32"][:])
        blob_fp8 = const_p.tile([128, n8], FP8, name="blob_fp8")
        nc.sync.dma_start(blob_fp8[:], dram["blob_fp8"][:])

        def cload(name):
            which, off, shape = lay[name]
            blob = {"bf": blob_bf, "f32": blob_f32, "fp8": blob_fp8}[which]
            w = int(np.prod(shape[1:]))
            ap = blob[0:shape[0], off:off + w]
            if len(shape) == 3:
                ap = ap.rearrange("p (a b) -> p a b", a=shape[1])
            return ap

        We1 = {(k, lp): cload(f"We1_{k}_{lp}")
               for k in range(3) for lp in range(4)}
        We = {l: [cload(f"We{l}_{k}") for k in range(3)] for l in (2, 3)}
        Ws1 = cload("Ws1")
        Ws2 = [cload(f"Ws2_{k}") for k in range(3)]
        Ws3 = [cload(f"Ws3_{k}") for k in range(3)]
        id128 = cload("id128")
        id128f = cload("id128f")
        bias = {n: cload(n) for n in
                ("bias_e1", "bias_e2", "bias_e3",
                 "bias_s1", "bias_s2", "bias_s3")}
        W2c = cload("W2c")
        SEL = cload("SEL")
        ones1x128 = cload("ones1x128")
        blin = cload("blin")

        out_sb = const_p.tile([1, BPC * S], F32, name="out_sb")
        inv_nx = const_p.tile([128, 1], F32, name="inv_nx")
        sqscr = const_p.tile([128, T], BF16, name="sqscr")

        xf = persist_p.tile([128, T], FP8, name="xf")
        xT2 = persist_p.tile([128, NCHK, 2, 128], FP8, name="xT2")

        evac_ctr = [0]

        phase = ["stim"]

        def evac_relu(dst, src, bias_t):
            # eeg phase: ACT handles psT copies, so conv evacs go to DVE;
            # stim phase: norms run on ACT, split evacs 3:2 DVE:ACT
            if phase[0] == "eeg":
                use_dve = evac_ctr[0] % 2 == 0
            else:
                use_dve = evac_ctr[0] % 3 < 2
            if use_dve:
                nc.vector.tensor_scalar(dst, src, bias_t[:, 0:1], 0.0,
                                        ALU.add, ALU.max)
            else:
                nc.scalar.activation(dst, src, AF.Relu, bias=bias_t[:, 0:1])
            evac_ctr[0] += 1

        def pair0(ap):
            """stride-0 k-tile pair view of a 2D ifmap slice"""
            v = ap.unsqueeze(1)
            v.ap[1] = [0, 2]
            return v

        def conv_dr(src_m, dst_m, out_len, dil, Wk, bn):
            # fp8 DoubleRow conv: per tap one DR matmul with (hi, lo) weight
            # tiles and a stride-0 ifmap pair; 0.5 cyc/row per tap.
            chs = _chunks(out_len)
            for i in range(0, len(chs), 2):
                grp = chs[i:i + 2]
                ps = psC_p.tile([128, 2 * CH], F32, name="psconv",
                                tag="psconv")
                for k in range(3):
                    for j, (t0, w) in enumerate(grp):
                        nc.tensor.matmul(
                            ps[:, j * CH:j * CH + w], Wk[k],
                            pair0(src_m[0:128,
                                        t0 + k * dil:t0 + k * dil + w]),
                            start=(k == 0), stop=(k == 2), perf_mode=DR)
                t0 = grp[0][0]
                wtot = CH + grp[1][1] if len(grp) == 2 else grp[0][1]
                evac_relu(dst_m[:, t0:t0 + wtot], ps[:, :wtot], bias[bn])

        def conv_bf16(src_m, dst_m, out_len, dil, Wk, bn):
            chs = _chunks(out_len)
            for i in range(0, len(chs), 2):
                grp = chs[i:i + 2]
                ps = psC_p.tile([128, 2 * CH], F32, name="psconv",
                                tag="psconv")
                for k in range(3):
                    for j, (t0, w) in enumerate(grp):
                        nc.tensor.matmul(
                            ps[:, j * CH:j * CH + w], Wk[k],
                            src_m[0:128, t0 + k * dil:t0 + k * dil + w],
                            start=(k == 0), stop=(k == 2))
                t0 = grp[0][0]
                wtot = CH + grp[1][1] if len(grp) == 2 else grp[0][1]
                evac_relu(dst_m[:, t0:t0 + wtot], ps[:, :wtot], bias[bn])

        # ---- early: stimulus group 0 s1+s2 (fills PE while eeg DMA runs)
        s1movs = [early_p.tile([24, T], BF16, name="s1mov",
                               tag=f"s1mov{i}") for i in range(2)]
        s2in = early_p.tile([128, L_E1], FP8, name="s2in")
        s3ins = [early_p.tile([128, L_E2], BF16, name="s3in",
                              tag=f"s3in{i}") for i in range(2)]

        def stim_s1(g):
            s1mov = s1movs[g % 2]
            for k in range(3):
                nc.gpsimd.dma_start(s1mov[k * 8:(k + 1) * 8, 0:L_E1],
                                    stim_in[g, :, k:k + L_E1])
            chs = _chunks(L_E1)
            for i in range(0, len(chs), 2):
                grp = chs[i:i + 2]
                ps = psC_p.tile([128, 2 * CH], F32, name="psconv",
                                tag="psconv")
                for j, (t0, w) in enumerate(grp):
                    nc.tensor.matmul(ps[:, j * CH:j * CH + w], Ws1[:],
                                     s1mov[0:24, t0:t0 + w])
                t0 = grp[0][0]
                wtot = CH + grp[1][1] if len(grp) == 2 else grp[0][1]
                evac_relu(s2in[:, t0:t0 + wtot], ps[:, :wtot],
                          bias["bias_s1"])

        stim_s1(0)
        conv_dr(s2in, s3ins[0], L_E2, 3, Ws2, "bias_s2")
        stim_s1(1)
        conv_dr(s2in, s3ins[1], L_E2, 3, Ws2, "bias_s2")

        # ================= EEG path =================
        with ExitStack() as ectx:
            phase[0] = "eeg"
            eeg_p = ectx.enter_context(tc.tile_pool(name="eegp", bufs=1))
            rot_p = ectx.enter_context(tc.tile_pool(name="eegrot", bufs=3))
            psT_p = ectx.enter_context(tc.tile_pool(name="psT", bufs=2,
                                                    space="PSUM"))

            e2in = eeg_p.tile([128, L_E1], FP8, name="e2in")
            e3in = eeg_p.tile([128, L_E2], FP8, name="e3in")

            TB = 4096
            chs_e1 = _chunks(L_E1)
            for duo in range(2):
                eegT = {}
                for lp in range(2):
                    p = 2 * duo + lp
                    eegT_p = eeg_p.tile([128, T], FP8, name="eegT",
                                        tag=f"eegT_{lp}")
                    eegT[lp] = eegT_p
                    for tb in range(T // TB):
                        ebf = rot_p.tile([128, TB // 128, 2, 64], BF16,
                                         name="ebf")
                        for dlt in range(2):
                            srcd = eeg_in[2 * p + dlt,
                                          tb * TB:(tb + 1) * TB, :]
                            nc.gpsimd.dma_start(
                                ebf[:, :, dlt, :],
                                srcd.rearrange("(th tl) c -> tl th c",
                                               tl=128))
                        for qb in range(TB // (2 * CH)):
                            psT = psT_p.tile([128, 8, 128], BF16,
                                             name="psT")
                            for u in range(8):
                                nc.tensor.matmul(psT[:, u, :],
                                                 ebf[:, qb * 8 + u, :, :],
                                                 id128[:],
                                                 is_transpose=True,
                                                 start=(u == 0),
                                                 stop=(u == 7))
                            t0 = tb * TB + qb * 2 * CH
                            nc.scalar.copy(
                                eegT_p[:, t0:t0 + 2 * CH], psT[:])
                # fused conv1x1+e1 in fp8 DR with padded 128-col weights;
                # lp outputs land at psum rows 32*(2*duo+lp)
                for i in range(0, len(chs_e1), 2):
                    grp = chs_e1[i:i + 2]
                    t0 = grp[0][0]
                    wtot = CH + grp[1][1] if len(grp) == 2 else grp[0][1]
                    ps = psC_p.tile([128, 2 * CH], F32, name="pse1",
                                    tag="psconv")
                    for lp in range(2):
                        for k in range(3):
                            for j, (tj, w) in enumerate(grp):
                                nc.tensor.matmul(
                                    ps[:, j * CH:j * CH + w],
                                    We1[(k, 2 * duo + lp)],
                                    pair0(eegT[lp][:, tj + k:tj + k + w]),
                                    start=(lp == 0 and k == 0),
                                    stop=(lp == 1 and k == 2),
                                    perf_mode=DR)
                    r0 = 64 * duo
                    evac_relu(e2in[r0:r0 + 64, t0:t0 + wtot],
                              ps[r0:r0 + 64, :wtot],
                              bias["bias_e1"][r0:r0 + 64])

            conv_dr(e2in, e3in, L_E2, 3, We[2], "bias_e2")
            conv_dr(e3in, xf, L_E3, 9, We[3], "bias_e3")
            phase[0] = "stim"

        # ================= stimulus path =================
        with ExitStack() as sctx:
            stim_p = sctx.enter_context(tc.tile_pool(name="stimp", bufs=1))
            stT_p = sctx.enter_context(tc.tile_pool(name="stTp", bufs=2))
            psD_p = sctx.enter_context(tc.tile_pool(name="psD", bufs=1,
                                                    space="PSUM"))
            psF_p = sctx.enter_context(tc.tile_pool(name="psF", bufs=1,
                                                    space="PSUM"))

            pending = []

            def emit_dot(g, stT, invns_row):
                dot_ps = psD_p.tile([128, 128], F32, name="dot_ps",
                                    tag="dot_ps")
                for c in range(NCHK):
                    rv = stT[:, c, :].bitcast(FP8).unsqueeze(1)
                    rv.ap[1] = [1, 2]
                    rv.ap[2] = [2, 128]
                    nc.tensor.matmul(dot_ps[:], xT2[:, c, :, :], rv,
                                     start=(c == 0), stop=(c == NCHK - 1),
                                     perf_mode=DR)
                # inv_ns broadcast over all partitions via two tiny matmuls
                f1 = const_p.tile([128, 128], F32, name="f1",
                                  tag=f"f1_{g % 2}")
                nc.vector.tensor_mul(f1[:], dot_ps[:], W2c[:])
                # psB reuses dot_ps's bank (WAR-ordered after the mul above)
                psB = psD_p.tile([128, 128], F32, name="psB", tag="dot_ps")
                nc.tensor.matmul(psB[:], ones1x128[:], invns_row[:])
                nc.vector.tensor_mul(f1[:], f1[:], psB[:])
                nc.vector.tensor_scalar_mul(f1[:], f1[:], inv_nx[:, 0:1])
                f3 = const_p.tile([128, S], F32, name="f3",
                                  tag=f"f3_{g % 2}")
                nc.vector.tensor_reduce(
                    f3[:], f1.rearrange("p (s i) -> p s i", i=16),
                    mybir.AxisListType.X, ALU.add)
                fin_ps = psF_p.tile([1, S], F32, name="fin_ps", tag="psN")
                nc.tensor.matmul(fin_ps[:], SEL[:, g:g + 1], f3[:])
                nc.vector.tensor_scalar_add(
                    out_sb[0:1, g * S:(g + 1) * S], fin_ps[:],
                    blin[0:1, 0:1])

            for g in range(BPC):
                st_cm = stim_p.tile([128, T], FP8, name="st_cm", bufs=2)
                nc.gpsimd.memset(st_cm[:, L_E3:T], 0.0)
                conv_bf16(s3ins[g % 2], st_cm, L_E3, 9, Ws3, "bias_s3")
                if g + 2 < BPC:
                    stim_s1(g + 2)
                    conv_dr(s2in, s3ins[g % 2], L_E2, 3, Ws2, "bias_s2")
                if g == 0:
                    # x norms from fp8 xf + packed-pair transpose + repack
                    nx2 = const_p.tile([128, 1], F32, name="nx2")
                    nx4 = const_p.tile([128, 4], F32, name="nx4")
                    qs = 2048
                    for q in range(4):
                        a, b = q * qs, min((q + 1) * qs, L_E3)
                        nc.scalar.activation(sqscr[:, a:b], xf[:, a:b],
                                             AF.Square,
                                             accum_out=nx4[:, q:q + 1])
                    nc.vector.tensor_reduce(nx2[:], nx4[:],
                                            mybir.AxisListType.X, ALU.add)
                    nc.scalar.sqrt(inv_nx[:], nx2[:])
                    nc.vector.tensor_scalar_max(inv_nx[:], inv_nx[:], EPS)
                    nc.vector.reciprocal(inv_nx[:], inv_nx[:])
                    nc.gpsimd.memset(xf[:, L_E3:T], 0.0)
                    xTu = stim_p.tile([128, NCHK, 128], U16, name="xTu")
                    for hh in range(2):
                        nc.sync.dma_start_transpose(
                            xTu[:, hh * 16:(hh + 1) * 16, :],
                            xf[:, hh * (T // 2):(hh + 1) * (T // 2)]
                            .bitcast(U16))
                    # repack byte pairs -> (even, odd) planes for ldweights
                    src = xTu[:].bitcast(FP8).unsqueeze(2)
                    src.ap[2] = [1, 2]
                    src.ap[3] = [2, 128]
                    nc.vector.tensor_copy(xT2[:], src)

                ns2 = const_p.tile([128, 1], F32, name="ns2",
                                   tag=f"ns2_{g % 2}")
                ns4 = const_p.tile([128, 4], F32, name="ns4",
                                   tag=f"ns4_{g % 2}")
                qs = 2048
                for q in range(4):
                    a, b = q * qs, min((q + 1) * qs, L_E3)
                    nc.scalar.activation(sqscr[:, a:b], st_cm[:, a:b],
                                         AF.Square,
                                         accum_out=ns4[:, q:q + 1])
                nc.vector.tensor_reduce(ns2[:], ns4[:],
                                        mybir.AxisListType.X, ALU.add)
                inv_ns = const_p.tile([128, 1], F32, name="inv_ns",
                                      tag=f"invns_{g % 2}")
                nc.scalar.sqrt(inv_ns[:], ns2[:])
                nc.vector.tensor_scalar_max(inv_ns[:], inv_ns[:], EPS)
                nc.vector.reciprocal(inv_ns[:], inv_ns[:])
                psN = psF_p.tile([1, 128], F32, name="psN", tag="psN")
                nc.tensor.matmul(psN[:], inv_ns[:], id128f[:],
                                 is_transpose=True)
                invns_row = const_p.tile([1, 128], F32, name="invns_row",
                                         tag=f"invrow_{g % 2}")
                nc.vector.tensor_copy(invns_row[:], psN[:])

                stT = stT_p.tile([128, NCHK, 128], U16, name="stT")
                for hh in range(2):
                    nc.sync.dma_start_transpose(
                        stT[:, hh * 16:(hh + 1) * 16, :],
                        st_cm[:, hh * (T // 2):(hh + 1) * (T // 2)]
                        .bitcast(U16))
                pending.append((g, stT, invns_row))
                if len(pending) > 1:
                    emit_dot(*pending.pop(0))
            while pending:
                emit_dot(*pending.pop(0))

        nc.sync.dma_start(out_dram[:], out_sb[:])


def _build(reps=1):
    nc = bass.Bass()
    dram = {
        "eeg_in": nc.dram_tensor("eeg_in", [BPC, T, C_EEG], F32,
                                 kind="ExternalInput"),
        "stim_in": nc.dram_tensor("stim_in", [BPC, S, T], F32,
                                  kind="ExternalInput"),
    }
    lay, nbf, nf, n8 = _blob_layout()
    dram["blob_bf"] = nc.dram_tensor("blob_bf", [128, nbf], BF16,
                                     kind="ExternalInput")
    dram["blob_f32"] = nc.dram_tensor("blob_f32", [128, nf], F32,
                                      kind="ExternalInput")
    dram["blob_fp8"] = nc.dram_tensor("blob_fp8", [128, n8], FP8,
                                      kind="ExternalInput")
    dram["out"] = nc.dram_tensor("out", [1, BPC * S], F32,
                                 kind="ExternalOutput")

    with tile.TileContext(nc) as tc:
        _build_body(nc, tc, dram)
    _split_multi_waits(nc)
    return nc


def _calib_scales(inp):
    """flat power-of-2 per-layer activation scales from a numpy calibration
    pass over a slice of the real inputs (outputs sampled strided)."""
    f32 = np.float32

    def conv_np(x, w, b, dil):
        K = w.shape[2]
        L = x.shape[2] - dil * (K - 1)
        out = np.zeros((x.shape[0], w.shape[0], L), f32)
        for k in range(K):
            out += np.einsum('oc,nct->not', w[:, :, k].astype(f32),
                             x[:, :, k * dil:k * dil + L])
        return np.maximum(out + b[None, :, None], 0)

    def pow2(x):
        return float(2.0 ** np.round(np.log2(max(x, 1e-30))))

    TGT = 64.0
    sl = np.s_[:, :, ::4]
    eeg = np.asarray(inp['eeg'], f32)
    stim = np.asarray(inp['stimulus'], f32)[..., 0]
    w_eeg = np.asarray(inp['w_eeg'], f32)
    w_e1 = np.asarray(inp['w_e1'], f32)
    Wf1 = np.einsum('ock,ci->oik', w_e1, w_eeg[:, :, 0])
    b_e1f = (np.asarray(inp['b_e1'], f32) +
             w_e1.sum(2) @ np.asarray(inp['b_eeg'], f32))
    g = lambda n: np.asarray(inp[n], f32)

    a = conv_np(eeg.transpose(0, 2, 1)[:4], Wf1, b_e1f, 1)
    S1 = pow2(TGT / (np.abs(a[sl]).max() + 1e-12))
    a = conv_np(a * S1, g('w_e2') / S1, g('b_e2'), 3)
    S2 = pow2(TGT / (np.abs(a[sl]).max() + 1e-12))
    a = conv_np(a * S2, g('w_e3') / S2, g('b_e3'), 9)
    S3 = pow2(TGT / (np.abs(a[sl]).max() + 1e-12))
    a = conv_np(stim.reshape(B * S, 1, T)[:8], g('w_s1'), g('b_s1'), 1)
    T1 = pow2(TGT / (np.abs(a[sl]).max() + 1e-12))
    a = conv_np(a * T1, g('w_s2') / T1, g('b_s2'), 3)
    T2 = pow2(TGT / (np.abs(a[sl]).max() + 1e-12))
    a = conv_np(a * T2, g('w_s3') / T2, g('b_s3'), 9)
    T3 = pow2(TGT / (np.abs(a[sl]).max() + 1e-12))
    return S1, S2, S3, T1, T2, T3


def _make_consts(inp):
    bf = ml_dtypes.bfloat16
    f8 = ml_dtypes.float8_e4m3fn
    f32 = np.float32
    S1, S2, S3, T1, T2, T3 = _calib_scales(inp)
    c = {}
    w_eeg = np.asarray(inp["w_eeg"], f32)      # [8, 64, 1]
    w_e1 = np.asarray(inp["w_e1"], f32)

    def hi_lo(m):
        """[rows, cols] f32 -> [rows, 2, cols] fp8 (hi, residual lo)"""
        hi = np.clip(m, -448, 448).astype(f8).astype(f32)
        lo = np.clip(m - hi, -448, 448).astype(f8)
        return np.stack([hi.astype(f8), lo], axis=1)

    def blockdiag(w, n_seq, ci, co):
        out = []
        for k in range(3):
            m = np.zeros((n_seq * ci, n_seq * co), f32)
            for s in range(n_seq):
                m[s * ci:(s + 1) * ci, s * co:(s + 1) * co] = w[:, :, k].T
            out.append(m)
        return out

    # fused conv1x1 + e1 (scale S1), per-lp padded to 128 cols
    for k in range(3):
        Mk = (w_e1[:, :, k] @ w_eeg[:, :, 0]) * S1   # [16 co, 64 c]
        base = np.zeros((128, 32), f32)
        for s in range(2):
            base[s * 64:(s + 1) * 64, s * 16:(s + 1) * 16] = Mk.T
        for lp in range(4):
            m = np.zeros((128, 128), f32)
            m[:, 32 * lp:32 * lp + 32] = base
            c[f"We1_{k}_{lp}"] = hi_lo(m)
    for l, wn, sc in ((2, "w_e2", S2 / S1), (3, "w_e3", S3 / S2)):
        mats = blockdiag(np.asarray(inp[wn], f32) * sc, 8, 16, 16)
        for k in range(3):
            c[f"We{l}_{k}"] = hi_lo(mats[k])
    mats = blockdiag(np.asarray(inp["w_s2"], f32) * (T2 / T1), 8, 16, 16)
    for k in range(3):
        c[f"Ws2_{k}"] = hi_lo(mats[k])
    mats = blockdiag(np.asarray(inp["w_s3"], f32) * (T3 / T2), 8, 16, 16)
    for k in range(3):
        c[f"Ws3_{k}"] = mats[k].astype(bf)
    w_s1 = np.asarray(inp["w_s1"], f32) * T1     # [16, 1, 3]
    Ws1 = np.zeros((24, 128), f32)
    for k in range(3):
        for s in range(8):
            Ws1[k * 8 + s, s * 16:(s + 1) * 16] = w_s1[:, 0, k]
    c["Ws1"] = Ws1.astype(bf)
    c["id128"] = np.eye(128, dtype=f32).astype(bf)
    c["id128f"] = np.eye(128, dtype=f32)
    b_e1f = (np.asarray(inp["b_e1"], f32) +
             sum(w_e1[:, :, k] for k in range(3))
             @ np.asarray(inp["b_eeg"], f32)) * S1
    c["bias_e1"] = np.tile(b_e1f, 8)[:, None]
    for n, srcn, sc in (("bias_e2", "b_e2", S2), ("bias_e3", "b_e3", S3),
                        ("bias_s1", "b_s1", T1), ("bias_s2", "b_s2", T2),
                        ("bias_s3", "b_s3", T3)):
        c[n] = (np.tile(np.asarray(inp[srcn], f32), 8) * sc)[:, None]
    w_lin = np.asarray(inp["w_lin"], f32).reshape(16, 16)  # [i, j]
    W2c = np.zeros((128, 128), f32)
    for gp in range(8):
        for s in range(8):
            W2c[gp * 16:(gp + 1) * 16, s * 16:(s + 1) * 16] = w_lin.T
    c["W2c"] = W2c
    SEL = np.zeros((128, 8), f32)
    for gp in range(8):
        SEL[gp * 16:(gp + 1) * 16, gp] = 1.0
    c["SEL"] = SEL
    c["ones1x128"] = np.ones((1, 128), f32)
    c["blin"] = np.full((128, 1), np.asarray(inp["b_lin"], f32).ravel()[0],
                        f32)
    lay, nbf, nf, n8 = _blob_layout()
    bb = np.zeros((128, nbf), bf)
    bf32 = np.zeros((128, nf), f32)
    b8 = np.zeros((128, n8), f8)
    for name, (which, off, shape) in lay.items():
        dst = {"bf": bb, "f32": bf32, "fp8": b8}[which]
        w = int(np.prod(shape[1:]))
        dst[0:shape[0], off:off + w] = \
            np.asarray(c[name]).reshape(shape[0], w)
    return {"blob_bf": bb, "blob_f32": bf32, "blob_fp8": b8}


def get_nc(reps=1):
    if reps not in _NC_CACHE:
        _NC_CACHE[reps] = _build(reps)
    return _NC_CACHE[reps]


def run(inputs, reps=1, trace=False):
    nc = get_nc(reps)
    consts = _make_consts(inputs)
    eeg = np.asarray(inputs["eeg"], np.float32)              # [64, 8192, 64]
    stim = np.asarray(inputs["stimulus"], np.float32)[..., 0]  # [64, 8, 8192]
    in_maps = []
    for ci in range(N_CORES):
        m = {"eeg_in": np.ascontiguousarray(eeg[ci * BPC:(ci + 1) * BPC]),
             "stim_in": np.ascontiguousarray(stim[ci * BPC:(ci + 1) * BPC])}
        m.update(consts)
        in_maps.append(m)
    res = run_bass_kernel_spmd(nc, in_maps, list(range(N_CORES)),
                               trace=trace)
    out = np.concatenate(
        [res.results[i]["out"].reshape(BPC, S) for i in range(N_CORES)],
        axis=0)
    return out.astype(np.float32)


def kernel(**inputs):
    return run(inputs, reps=1)
